# revision 6
# baseline (speedup 1.0000x reference)
"""Masked dot-product attention (B=16, S=4096, D=64) on 8 Trainium2 NeuronCores.

Decomposition: query-block sharding. Core c owns query rows [c*512, (c+1)*512)
of ALL batches. Every core runs the identical SPMD program: for each batch b it
loops over exactly kb[b] = ceil(valid_lens[b]/128) key-chunks (compile-time
constants derived from the valid_lens input on the host), so masked-out key
blocks are never computed and the load is perfectly balanced across cores.

Math (per batch b, per core c):
  S^T[k,q] = K_chunk[k,:] @ Q[q,:]^T / sqrt(D)      (TensorE, bf16, k on partitions)
  P^T      = exp(S^T)                               (ScalarE, no max-subtraction:
                                                     scores ~ N(0,1), no overflow)
  Oaug^T[65,q] += V_aug_chunk[k,:]^T @ P^T[k,q]     (TensorE, accumulate in PSUM)
where V_aug = [V | 1], with rows k >= valid_len zeroed on the host. The zeroed
rows make masking exact: masked keys contribute 0 to both the numerator and the
ones-column denominator. Host divides numerator by denominator at the end
(exactly softmax @ V, since exp(-1e6 + s) underflows to 0 in fp32 in the
reference as well).
"""

import numpy as np
import ml_dtypes

import concourse.bacc as bacc
import concourse.tile as tile
from concourse import mybir
from concourse.bass_utils import run_bass_kernel_spmd

F16 = np.float16
F32 = np.float32

NCORES = 8
CH = 128   # key-chunk size (PSUM/PE partition dim)
EW = 65    # V_aug width: 64 value dims + 1 ones-column (softmax denominator)
# exp() groups alternate 4-chunk / 3-chunk tiles (7 PSUM banks total, ping-pong
# at tag granularity) + 1 bank for the PV accumulator = 8 banks exactly.
GRP_A, GRP_B = 4, 3


def _schedule(valid_lens, S):
    vl = np.asarray(valid_lens).astype(np.int64)
    vl = np.clip(vl, 1, S)
    kb = [int(-(-int(x) // CH)) for x in vl]          # ceil(valid/CH), >= 1
    pairs = [(x + 1) // 2 for x in kb]
    return vl, kb, pairs


def _build_program(kb, pairs, B, QB, D, slot_order):
    """Emit the SPMD Tile program. Identical on all cores; per-core data differs.

    Emission is software-pipelined one exp-group ahead: the PE's program order
    is S_0, S_1, PV_0, S_2, PV_1, ... so the scores for group j+1 are already
    in PSUM when exp(j) finishes — ScalarE (the bottleneck engine) never waits
    at slot boundaries.
    """
    TOT = sum(kb)
    TP = sum(pairs)
    dt = mybir.dt
    nc = bacc.Bacc(None, target_bir_lowering=False)

    kt2 = nc.declare_dram_parameter("kt2", [128, TP * CH], dt.float16, False)
    va = nc.declare_dram_parameter("va", [128, TOT * EW], dt.float16, False)
    qt2 = nc.declare_dram_parameter("qt2", [128, B * QB], dt.float16, False)
    oaug = nc.declare_dram_parameter("oaug", [B, EW, QB], dt.float32, True)

    poffs = np.concatenate([[0], np.cumsum(pairs)])
    coffs = np.concatenate([[0], np.cumsum(kb)])

    # flat group list with globally alternating 4/3 tags
    groups = []  # (b, [chunk indices], tag, first_of_slot, last_of_slot)
    for b in slot_order:
        rem = list(range(kb[b]))
        first = True
        while rem:
            size = GRP_A if len(groups) % 2 == 0 else GRP_B
            take, rem = rem[:size], rem[size:]
            groups.append((b, take, len(groups) % 2, first, not rem))
            first = False

    with tile.TileContext(nc) as tc:
        with (
            tc.tile_pool(name="ins", bufs=1) as ins,
            tc.tile_pool(name="ptp", bufs=3) as ptp,
            tc.tile_pool(name="obp", bufs=3) as obp,
            tc.tile_pool(name="sca", bufs=1, space="PSUM") as scap,
            tc.tile_pool(name="scb", bufs=1, space="PSUM") as scbp,
            tc.tile_pool(name="acp", bufs=1, space="PSUM") as acp,
        ):
            kts, vas, qts = {}, {}, {}
            dma_engines = [nc.sync, nc.gpsimd]
            for i, b in enumerate(slot_order):
                eng = dma_engines[i % len(dma_engines)]
                kt_t = ins.tile([128, pairs[b] * CH], dt.float16, tag=f"kt{b}")
                eng.dma_start(
                    out=kt_t[:],
                    in_=kt2[:, int(poffs[b]) * CH:int(poffs[b + 1]) * CH],
                )
                va_t = ins.tile([128, kb[b] * EW], dt.float16, tag=f"va{b}")
                eng.dma_start(
                    out=va_t[:],
                    in_=va[:, int(coffs[b]) * EW:int(coffs[b + 1]) * EW],
                )
                qt_t = ins.tile([128, QB], dt.float16, tag=f"qt{b}")
                eng.dma_start(out=qt_t[:], in_=qt2[:, b * QB:(b + 1) * QB])
                kts[b], vas[b], qts[b] = kt_t, va_t, qt_t

            accs = {}

            def emit_scores(b, chunks, tag):
                cap = GRP_A if tag == 0 else GRP_B
                pool = scap if tag == 0 else scbp
                sc = pool.tile([128, cap, QB], dt.float32, tag="sc")
                for i, ci in enumerate(chunks):
                    pj, par = divmod(ci, 2)
                    lhsT = kts[b][par * 64:(par + 1) * 64, pj * CH:(pj + 1) * CH]
                    rhs = qts[b][par * 64:(par + 1) * 64, :]
                    nc.tensor.matmul(
                        sc[:, i, :], lhsT, rhs,
                        start=True, stop=True,
                        tile_position=(par * 64, 0),
                    )
                return sc

            def emit_pv(b, chunks, pt, last_of_slot):
                if chunks[0] == 0:
                    accs[b] = acp.tile([128, QB], dt.float32, tag="acc", name="acc")
                for i, ci in enumerate(chunks):
                    nc.tensor.matmul(
                        accs[b][0:EW, :],
                        vas[b][:, ci * EW:(ci + 1) * EW],
                        pt[:, i, :],
                        start=(ci == 0),
                        stop=(ci == kb[b] - 1),
                    )
                if last_of_slot:
                    ob = obp.tile([128, QB], dt.float32, tag="ob")
                    nc.vector.tensor_copy(ob[0:EW, :], accs[b][0:EW, :])
                    nc.sync.dma_start(out=oaug[b], in_=ob[0:EW, :])

            prev = None  # (b, chunks, pt, last_of_slot)
            for b, chunks, tag, first, last in groups:
                sc = emit_scores(b, chunks, tag)
                if prev is not None:
                    emit_pv(prev[0], prev[1], prev[2], prev[3])
                n = len(chunks)
                cap = GRP_A if tag == 0 else GRP_B
                pt = ptp.tile([128, GRP_A, QB], dt.float16, tag="pt")
                nc.scalar.activation(
                    pt[:, :n, :], sc[:, :n, :],
                    mybir.ActivationFunctionType.Exp,
                    scale=float(1.0 / np.sqrt(D)),
                )
                prev = (b, chunks, pt, last)
            emit_pv(prev[0], prev[1], prev[2], prev[3])

    nc.compile()
    return nc


def _prepare(q, k, v, valid_lens):
    """Host-side sharding/layout. Returns (nc, in_maps, meta)."""
    q = np.asarray(q, dtype=F32)
    k = np.asarray(k, dtype=F32)
    v = np.asarray(v, dtype=F32)
    B, S, D = q.shape
    QB = S // NCORES
    vl, kb, pairs = _schedule(valid_lens, S)
    TOT, TP = sum(kb), sum(pairs)

    # kt2: [128, TP*CH] bf16. Pair j of batch b: partitions 0:64 <- K^T chunk 2j,
    # partitions 64:128 <- K^T chunk 2j+1 (left zero if absent). Concurrent
    # row-group matmuls on the PE use both halves of the systolic array.
    kT = np.ascontiguousarray(k.transpose(0, 2, 1)).astype(F16)  # [B, D, S]
    kt2 = np.zeros((128, TP * CH), dtype=F16)
    poff = 0
    for b in range(B):
        for j in range(pairs[b]):
            c0, c1 = 2 * j, 2 * j + 1
            kt2[0:64, (poff + j) * CH:(poff + j + 1) * CH] = \
                kT[b][:, c0 * CH:(c0 + 1) * CH]
            if c1 < kb[b]:
                kt2[64:128, (poff + j) * CH:(poff + j + 1) * CH] = \
                    kT[b][:, c1 * CH:(c1 + 1) * CH]
        poff += pairs[b]

    # va: [128, TOT*EW] bf16. Chunk g of batch b at columns (coff+g)*EW:
    # va[p, (coff+g)*EW + e] = V_aug[b, g*CH + p, e], rows >= valid zeroed.
    va_aug = np.zeros((B, S, EW), dtype=F32)
    va_aug[:, :, :D] = v
    va_aug[:, :, D] = 1.0
    for b in range(B):
        va_aug[b, int(vl[b]):, :] = 0.0
    va_aug = va_aug.astype(F16)
    va = np.zeros((128, TOT * EW), dtype=F16)
    coff = 0
    for b in range(B):
        blk = va_aug[b, :kb[b] * CH, :].reshape(kb[b], CH, EW)
        va[:, coff * EW:(coff + kb[b]) * EW] = \
            blk.transpose(1, 0, 2).reshape(CH, kb[b] * EW)
        coff += kb[b]

    # qt2 (per core): [128, B*QB] bf16, Q^T slice duplicated on both partition
    # halves (each PE row-group streams its own rhs copy).
    qT = np.ascontiguousarray(q.transpose(0, 2, 1)).astype(F16)  # [B, D, S]
    in_maps = []
    for c in range(NCORES):
        qt2 = np.zeros((128, B * QB), dtype=F16)
        for b in range(B):
            sl = qT[b][:, c * QB:(c + 1) * QB]
            qt2[0:64, b * QB:(b + 1) * QB] = sl
            qt2[64:128, b * QB:(b + 1) * QB] = sl
        in_maps.append({"kt2": kt2, "va": va, "qt2": qt2})

    slot_order = sorted(range(B), key=lambda b: kb[b])  # smallest first: fast start
    nc = _build_program(kb, pairs, B, QB, D, slot_order)
    return nc, in_maps, (B, S, D, QB)


def _postprocess(results, meta):
    B, S, D, QB = meta
    out = np.empty((B, S, D), dtype=F32)
    for c in range(NCORES):
        oa = results[c]["oaug"]          # [B, EW, QB] f32
        num = oa[:, :D, :]
        den = oa[:, D:D + 1, :]
        out[:, c * QB:(c + 1) * QB, :] = (num / den).transpose(0, 2, 1)
    return out


def kernel(q, k, v, valid_lens):
    nc, in_maps, meta = _prepare(q, k, v, valid_lens)
    res = run_bass_kernel_spmd(nc, in_maps, list(range(NCORES)))
    return _postprocess(res.results, meta)


# revision 7
# speedup vs baseline: 1.1091x; 1.1091x over previous
"""Masked dot-product attention (B=16, S=4096, D=64) on 8 Trainium2 NeuronCores.

Decomposition: query-block sharding. Core c owns query rows [c*512, (c+1)*512)
of ALL batches. Every core runs the identical SPMD program: for each batch b it
loops over exactly kb[b] = ceil(valid_lens[b]/128) key-chunks (compile-time
constants derived from the valid_lens input on the host), so masked-out key
blocks are never computed and the load is perfectly balanced across cores.

Math (per batch b, per core c):
  S^T[k,q] = K_chunk[k,:] @ Q[q,:]^T / sqrt(D)      (TensorE, bf16, k on partitions)
  P^T      = exp(S^T)                               (ScalarE, no max-subtraction:
                                                     scores ~ N(0,1), no overflow)
  Oaug^T[65,q] += V_aug_chunk[k,:]^T @ P^T[k,q]     (TensorE, accumulate in PSUM)
where V_aug = [V | 1], with rows k >= valid_len zeroed on the host. The zeroed
rows make masking exact: masked keys contribute 0 to both the numerator and the
ones-column denominator. Host divides numerator by denominator at the end
(exactly softmax @ V, since exp(-1e6 + s) underflows to 0 in fp32 in the
reference as well).
"""

import numpy as np
import ml_dtypes

import concourse.bacc as bacc
import concourse.tile as tile
from concourse import mybir
from concourse.bass_utils import run_bass_kernel_spmd

F16 = np.float16
F32 = np.float32

NCORES = 8
CH = 128   # key-chunk size (PSUM/PE partition dim)
EW = 65    # V_aug width: 64 value dims + 1 ones-column (softmax denominator)
# exp() groups alternate 4-chunk / 3-chunk tiles (7 PSUM banks total, ping-pong
# at tag granularity) + 1 bank for the PV accumulator = 8 banks exactly.
GRP_A, GRP_B = 4, 3


def _schedule(valid_lens, S):
    vl = np.asarray(valid_lens).astype(np.int64)
    vl = np.clip(vl, 1, S)
    kb = [int(-(-int(x) // CH)) for x in vl]          # ceil(valid/CH), >= 1
    pairs = [(x + 1) // 2 for x in kb]
    return vl, kb, pairs


def _build_program(kb, pairs, B, QB, D, slot_order):
    """Emit the SPMD Tile program. Identical on all cores; per-core data differs.

    Emission is software-pipelined one exp-group ahead: the PE's program order
    is S_0, S_1, PV_0, S_2, PV_1, ... so the scores for group j+1 are already
    in PSUM when exp(j) finishes — ScalarE (the bottleneck engine) never waits
    at slot boundaries.
    """
    TOT = sum(kb)
    TP = sum(pairs)
    dt = mybir.dt
    nc = bacc.Bacc(None, target_bir_lowering=False)

    kt2 = nc.declare_dram_parameter("kt2", [128, TP * CH], dt.float16, False)
    va = nc.declare_dram_parameter("va", [128, TOT * EW], dt.float16, False)
    qt2 = nc.declare_dram_parameter("qt2", [128, B * QB], dt.float16, False)
    oaug = nc.declare_dram_parameter("oaug", [B, EW, QB], dt.float32, True)

    poffs = np.concatenate([[0], np.cumsum(pairs)])
    coffs = np.concatenate([[0], np.cumsum(kb)])

    # flat group list with globally alternating 4/3 tags
    groups = []  # (b, [chunk indices], tag, first_of_slot, last_of_slot)
    for b in slot_order:
        rem = list(range(kb[b]))
        first = True
        while rem:
            size = GRP_A if len(groups) % 2 == 0 else GRP_B
            take, rem = rem[:size], rem[size:]
            groups.append((b, take, len(groups) % 2, first, not rem))
            first = False

    with tile.TileContext(nc) as tc:
        with (
            tc.tile_pool(name="ins", bufs=1) as ins,
            tc.tile_pool(name="ptp", bufs=3) as ptp,
            tc.tile_pool(name="obp", bufs=3) as obp,
            tc.tile_pool(name="sca", bufs=1, space="PSUM") as scap,
            tc.tile_pool(name="scb", bufs=1, space="PSUM") as scbp,
            tc.tile_pool(name="acp", bufs=1, space="PSUM") as acp,
        ):
            kts, vas, qts = {}, {}, {}
            dma_engines = [nc.sync, nc.gpsimd]
            for i, b in enumerate(slot_order):
                eng = dma_engines[i % len(dma_engines)]
                kt_t = ins.tile([128, pairs[b] * CH], dt.float16, tag=f"kt{b}")
                eng.dma_start(
                    out=kt_t[:],
                    in_=kt2[:, int(poffs[b]) * CH:int(poffs[b + 1]) * CH],
                )
                va_t = ins.tile([128, kb[b] * EW], dt.float16, tag=f"va{b}")
                eng.dma_start(
                    out=va_t[:],
                    in_=va[:, int(coffs[b]) * EW:int(coffs[b + 1]) * EW],
                )
                qt_t = ins.tile([128, QB], dt.float16, tag=f"qt{b}")
                eng.dma_start(out=qt_t[:], in_=qt2[:, b * QB:(b + 1) * QB])
                kts[b], vas[b], qts[b] = kt_t, va_t, qt_t

            accs = {}

            def emit_scores(b, chunks, tag):
                cap = GRP_A if tag == 0 else GRP_B
                pool = scap if tag == 0 else scbp
                sc = pool.tile([128, cap, QB], dt.float32, tag="sc")
                for i, ci in enumerate(chunks):
                    pj, par = divmod(ci, 2)
                    lhsT = kts[b][par * 64:(par + 1) * 64, pj * CH:(pj + 1) * CH]
                    rhs = qts[b][par * 64:(par + 1) * 64, :]
                    nc.tensor.matmul(
                        sc[:, i, :], lhsT, rhs,
                        start=True, stop=True,
                        tile_position=(par * 64, 0),
                    )
                return sc

            def emit_pv(b, chunks, pt, last_of_slot):
                if chunks[0] == 0:
                    accs[b] = acp.tile([128, QB], dt.float32, tag="acc", name="acc")
                for i, ci in enumerate(chunks):
                    nc.tensor.matmul(
                        accs[b][0:EW, :],
                        vas[b][:, ci * EW:(ci + 1) * EW],
                        pt[:, i, :],
                        start=(ci == 0),
                        stop=(ci == kb[b] - 1),
                    )
                if last_of_slot:
                    ob = obp.tile([128, QB], dt.float32, tag="ob")
                    nc.vector.tensor_copy(ob[0:EW, :], accs[b][0:EW, :])
                    nc.sync.dma_start(out=oaug[b], in_=ob[0:EW, :])

            prev = None  # (b, chunks, pt, last_of_slot)
            for b, chunks, tag, first, last in groups:
                sc = emit_scores(b, chunks, tag)
                if prev is not None:
                    emit_pv(prev[0], prev[1], prev[2], prev[3])
                n = len(chunks)
                cap = GRP_A if tag == 0 else GRP_B
                pt = ptp.tile([128, GRP_A, QB], dt.float16, tag="pt")
                nc.scalar.activation(
                    pt[:, :n, :], sc[:, :n, :],
                    mybir.ActivationFunctionType.Exp,
                    scale=float(1.0 / np.sqrt(D)),
                )
                prev = (b, chunks, pt, last)
            emit_pv(prev[0], prev[1], prev[2], prev[3])

    nc.compile()
    return nc


def _prepare(q, k, v, valid_lens):
    """Host-side sharding/layout. Returns (nc, in_maps, meta)."""
    q = np.asarray(q, dtype=F32)
    k = np.asarray(k, dtype=F32)
    v = np.asarray(v, dtype=F32)
    B, S, D = q.shape
    QB = S // NCORES
    vl, kb, pairs = _schedule(valid_lens, S)
    TOT, TP = sum(kb), sum(pairs)

    # kt2: [128, TP*CH] bf16. Pair j of batch b: partitions 0:64 <- K^T chunk 2j,
    # partitions 64:128 <- K^T chunk 2j+1 (left zero if absent). Concurrent
    # row-group matmuls on the PE use both halves of the systolic array.
    kT = np.ascontiguousarray(k.transpose(0, 2, 1)).astype(F16)  # [B, D, S]
    kt2 = np.zeros((128, TP * CH), dtype=F16)
    poff = 0
    for b in range(B):
        for j in range(pairs[b]):
            c0, c1 = 2 * j, 2 * j + 1
            kt2[0:64, (poff + j) * CH:(poff + j + 1) * CH] = \
                kT[b][:, c0 * CH:(c0 + 1) * CH]
            if c1 < kb[b]:
                kt2[64:128, (poff + j) * CH:(poff + j + 1) * CH] = \
                    kT[b][:, c1 * CH:(c1 + 1) * CH]
        poff += pairs[b]

    # va: [128, TOT*EW] bf16. Chunk g of batch b at columns (coff+g)*EW:
    # va[p, (coff+g)*EW + e] = V_aug[b, g*CH + p, e], rows >= valid zeroed.
    va_aug = np.zeros((B, S, EW), dtype=F32)
    va_aug[:, :, :D] = v
    va_aug[:, :, D] = 1.0
    for b in range(B):
        va_aug[b, int(vl[b]):, :] = 0.0
    va_aug = va_aug.astype(F16)
    va = np.zeros((128, TOT * EW), dtype=F16)
    coff = 0
    for b in range(B):
        blk = va_aug[b, :kb[b] * CH, :].reshape(kb[b], CH, EW)
        va[:, coff * EW:(coff + kb[b]) * EW] = \
            blk.transpose(1, 0, 2).reshape(CH, kb[b] * EW)
        coff += kb[b]

    # qt2 (per core): [128, B*QB] bf16, Q^T slice duplicated on both partition
    # halves (each PE row-group streams its own rhs copy).
    qT = np.ascontiguousarray(q.transpose(0, 2, 1)).astype(F16)  # [B, D, S]
    in_maps = []
    for c in range(NCORES):
        qt2 = np.zeros((128, B * QB), dtype=F16)
        for b in range(B):
            sl = qT[b][:, c * QB:(c + 1) * QB]
            qt2[0:64, b * QB:(b + 1) * QB] = sl
            qt2[64:128, b * QB:(b + 1) * QB] = sl
        in_maps.append({"kt2": kt2, "va": va, "qt2": qt2})

    # Smallest slot first (its tiny DMAs gate the very first exp), then the
    # rest descending: big slots early keep compute behind the DMA stream.
    asc = sorted(range(B), key=lambda b: kb[b])
    slot_order = [asc[0]] + asc[:0:-1]
    nc = _build_program(kb, pairs, B, QB, D, slot_order)
    return nc, in_maps, (B, S, D, QB)


def _postprocess(results, meta):
    B, S, D, QB = meta
    out = np.empty((B, S, D), dtype=F32)
    for c in range(NCORES):
        oa = results[c]["oaug"]          # [B, EW, QB] f32
        num = oa[:, :D, :]
        den = oa[:, D:D + 1, :]
        out[:, c * QB:(c + 1) * QB, :] = (num / den).transpose(0, 2, 1)
    return out


def kernel(q, k, v, valid_lens):
    nc, in_maps, meta = _prepare(q, k, v, valid_lens)
    res = run_bass_kernel_spmd(nc, in_maps, list(range(NCORES)))
    return _postprocess(res.results, meta)
